# revision 5
# baseline (speedup 1.0000x reference)
"""Trainium2 Bass kernel for ive(63, z) = exp(-z) * I_63(z), elementwise over z[1048576].

Math: uniform asymptotic (Debye) expansion of log I_v(z) for fixed v=63:
    r = sqrt(z^2 + v^2),  Ls = ln(z^2 + v^2)
    log_ive = (r - z) + v*ln(z) - v*ln(v + r) - 0.25*Ls - 0.5*ln(2*pi) + q(Ls)
where q(Ls) ~= ln(1 + u1(t)/v + u2(t)/v^2), t = v*exp(-Ls/2), fit offline as a
cubic in Ls (max abs err 3.9e-5; the f32 reference's own noise vs truth is ~1e-4).

Sharding: pure elementwise; z split into 8 contiguous chunks of 131072 = [128,1024],
one per NeuronCore, no communication. Output flushed to 0 below FLT_MIN to match
the XLA reference (which produces no subnormals).
"""
import numpy as np
from contextlib import ExitStack

N = 1048576
NCORES = 8
P = 128
FD_CORE = 1024          # 128*1024 = 131072 elements per core
CHUNK = 512             # free-dim chunk per instruction
V2 = 3969.0             # 63^2

# q(Ls) cubic fit coeffs (power basis, c0..c3), fit over z in [0.1, 200]
C0 = -0.35607278238618795
C1 = 0.1060258802108635
C2 = -0.010491346839964302
C3 = 0.00034552829416086543
BIAS = C0 - 0.9189385332046727   # c0 + (-0.5*ln(2*pi)), folded into (r - z) op
C1F = C1 - 0.25                  # fold -0.25*Ls into the poly's linear term
FLT_MIN = 1.1754944e-38

_CACHE = {}


def _build():
    import concourse.tile as tile
    import concourse.mybir as mybir
    from concourse import bacc

    F32 = mybir.dt.float32
    AF = mybir.ActivationFunctionType
    ALU = mybir.AluOpType

    nc = bacc.Bacc("TRN2", target_bir_lowering=False, debug=False)
    # const APs for activation biases
    for v in (V2, 63.0, BIAS):
        t = nc.alloc_sbuf_tensor(f"constf32-{v}", [128, 1], F32)
        nc.gpsimd.memset(t.ap(), float(v))
        nc.const_aps.aps[(F32, float(v))] = t.ap()
    nc.all_engine_barrier()

    z = nc.dram_tensor("z", [P, FD_CORE], F32, kind="ExternalInput").ap()
    out = nc.dram_tensor("out", [P, FD_CORE], F32, kind="ExternalOutput").ap()

    with tile.TileContext(nc) as tc, ExitStack() as ctx:
        pool = ctx.enter_context(tc.tile_pool(name="sbuf", bufs=3))
        import concourse.bass as bass

        for i in range(FD_CORE // CHUNK):
            sl = bass.ts(i, CHUNK)
            zt = pool.tile([P, CHUNK], F32)
            nc.sync.dma_start(zt[:], z[:, sl])

            zsq = pool.tile([P, CHUNK], F32)
            nc.vector.tensor_tensor(zsq[:], zt[:], zt[:], ALU.mult)

            Ls = pool.tile([P, CHUNK], F32)
            nc.scalar.activation(Ls[:], zsq[:], AF.Ln, bias=V2, scale=1.0)
            r = pool.tile([P, CHUNK], F32)
            nc.scalar.activation(r[:], Ls[:], AF.Exp, bias=0.0, scale=0.5)
            L1 = pool.tile([P, CHUNK], F32)
            nc.scalar.activation(L1[:], zt[:], AF.Ln, bias=0.0, scale=1.0)
            L2 = pool.tile([P, CHUNK], F32)
            nc.scalar.activation(L2[:], r[:], AF.Ln, bias=63.0, scale=1.0)

            # q(Ls) + (-0.25*Ls): q0=C3*Ls; q1=(q0+C2)*Ls; q2=(q1+C1F)*Ls  (c0 -> BIAS)
            q0 = pool.tile([P, CHUNK], F32)
            nc.vector.tensor_scalar(q0[:], Ls[:], C3, None, ALU.mult)
            q1 = pool.tile([P, CHUNK], F32)
            nc.vector.scalar_tensor_tensor(q1[:], q0[:], C2, Ls[:], ALU.add, ALU.mult)
            q2 = pool.tile([P, CHUNK], F32)
            nc.vector.scalar_tensor_tensor(q2[:], q1[:], C1F, Ls[:], ALU.add, ALU.mult)

            # a = L1 - L2 (gpsimd); b = 63*a + q2; c = (r + BIAS) - z (gpsimd); g = b + c
            a = pool.tile([P, CHUNK], F32)
            nc.gpsimd.tensor_tensor(a[:], L1[:], L2[:], ALU.subtract)
            b = pool.tile([P, CHUNK], F32)
            nc.vector.scalar_tensor_tensor(b[:], a[:], 63.0, q2[:], ALU.mult, ALU.add)
            c = pool.tile([P, CHUNK], F32)
            nc.gpsimd.tensor_tensor(c[:], r[:], zt[:], ALU.subtract)
            g = pool.tile([P, CHUNK], F32)
            nc.vector.tensor_tensor(g[:], b[:], c[:], ALU.add)

            e = pool.tile([P, CHUNK], F32)
            nc.scalar.activation(e[:], g[:], AF.Exp, bias=BIAS, scale=1.0)
            # flush subnormals to 0 to match XLA: (e >= FLT_MIN) * e
            o = pool.tile([P, CHUNK], F32)
            nc.vector.scalar_tensor_tensor(o[:], e[:], FLT_MIN, e[:], ALU.is_ge, ALU.mult)

            nc.sync.dma_start(out[:, sl], o[:])

    nc.compile()
    return nc


def kernel(z: np.ndarray) -> np.ndarray:
    from concourse.bass_utils import run_bass_kernel_spmd

    if "nc" not in _CACHE:
        _CACHE["nc"] = _build()
    nc = _CACHE["nc"]

    z = np.ascontiguousarray(z, dtype=np.float32)
    zs = z.reshape(NCORES, P, FD_CORE)
    in_maps = [{"z": zs[i]} for i in range(NCORES)]
    res = run_bass_kernel_spmd(nc, in_maps, core_ids=list(range(NCORES)))
    out = np.stack([r["out"] for r in res.results])
    return out.reshape(N).astype(np.float32)
